# revision 31
# baseline (speedup 1.0000x reference)
"""Trainium2 Bass kernel for space-to-depth (pixel-unshuffle, factor 2).

Input  x:   (8, 32, 512, 512) f32
Output out: (8, 128, 256, 256) f32 with out[b, 4i+2dh+dw, h, w] = x[b, i, 2h+dh, 2w+dw]

Sharding: data-parallel over batch -- core b processes sample b (no comms).

Per-core dataflow (v2): 8 groups of 4 input channels (4MB tiles).
  - one 4MB load per group (32KB contiguous per partition; SP HWDGE ring)
  - 4 strided DVE copies (one per (dh,dw) phase) deinterleave in SBUF;
    fully hidden behind DMA
  - 4 stores of 1MB per group (8KB contiguous runs; ACT HWDGE ring;
    single_packet=True bundles each engine's descriptors into one packet,
    coarsening R/W interleave -- won 3/4 paired A/Bs, mean +4.6us/op)
Measured on TRN2 (8 cores): ~203-210us exec; steady-state marginal rate
~190us/op = 354 GB/s combined R+W per core (pure-read 399, pure-write 374,
interleaved-mix 313 GB/s measured), ~8us framework preamble + ~3us tail.
"""

import numpy as np

from concourse import bacc, mybir, tile
from concourse.bass_utils import run_bass_kernel_spmd

B, C, H, W = 8, 32, 512, 512
N_CORES = 8

_cache = {}


def _build_nc(finalize=True, reps=1, variant="v3", bufs=None, store_engine="scalar", store_sp=True, load_sp=False, store_run=16384):
    if bufs is None:
        bufs = (2, 1) if variant == "v7" else (2, 2) if variant in ("v3", "v4", "v5", "v8", "v9") else (3, 2)
    nc = bacc.Bacc(
        "TRN2", target_bir_lowering=False, debug=False, num_devices=N_CORES
    )
    x = nc.dram_tensor("x", [C, H, W], mybir.dt.float32, kind="ExternalInput")
    out = nc.dram_tensor(
        "out", [4 * C, H // 2, W // 2], mybir.dt.float32, kind="ExternalOutput"
    )
    xa, oa = x.ap(), out.ap()

    if variant == "raw":
        _emit_raw(nc, xa, oa, reps)
    else:
        with tile.TileContext(nc) as tc:
            if variant == "v1":
                _emit_v1(nc, tc, xa, oa, reps)
            elif variant == "v3":
                _emit_v3(nc, tc, xa, oa, reps, bufs, store_engine, store_sp, load_sp, store_run)
            elif variant == "v4":
                _emit_v4(nc, tc, xa, oa, reps, bufs, store_engine, store_sp, load_sp, store_run)
            elif variant == "v5":
                _emit_v5(nc, tc, xa, oa, reps, bufs, store_engine, store_sp, load_sp)
            elif variant == "v7":
                _emit_v7(nc, tc, xa, oa, reps, bufs, store_engine, store_sp, load_sp)
            elif variant == "v8":
                _emit_v8(nc, tc, xa, oa, reps, bufs, store_engine, store_sp, load_sp)
            elif variant == "v9":
                _emit_v9(nc, tc, xa, oa, reps, bufs, store_engine, store_sp, load_sp)
            elif variant.startswith("cal_"):
                _emit_cal(nc, tc, xa, oa, reps, variant[4:])
            else:
                _emit_v2(nc, tc, xa, oa, reps, bufs, store_engine, store_sp, load_sp)
    if finalize:
        nc.finalize()
    return nc


def _emit_raw(nc, xa, oa, reps):
    """Same dataflow as v2 but raw bacc: hand-rolled semaphore pipeline,
    no TileContext, so the first load issues immediately instead of after
    the ~8us Tile preamble.  3 in-tile buffers, 2 staging buffers.
    """
    G = C // 4
    NB_IN, NB_ST = 3, 2
    tin = [
        nc.alloc_sbuf_tensor(f"tin{j}", [128, 8192], mybir.dt.float32)
        for j in range(NB_IN)
    ]
    tst = [
        nc.alloc_sbuf_tensor(f"tst{j}", [128, 8192], mybir.dt.float32)
        for j in range(NB_ST)
    ]
    n = G * reps
    from contextlib import ExitStack

    with ExitStack() as ctx:
        block = ctx.enter_context(nc.Block())
        # per-buffer sem rotation so concurrent DMAs never share a semaphore
        ld_sems = [
            ctx.enter_context(nc.semaphore(f"ld_sem{j}")) for j in range(NB_IN)
        ]
        st_sems = [
            ctx.enter_context(nc.semaphore(f"st_sem{j}")) for j in range(NB_ST)
        ]
        cp_sem = ctx.enter_context(nc.semaphore("cp_sem"))

        @block.sync
        def _(sync):
            for k in range(n):
                g = k % G
                if k >= NB_IN:
                    # in-buffer reuse: copies of group k-NB_IN must be done
                    sync.wait_ge(cp_sem, k - NB_IN + 1)
                sync.dma_start(
                    tin[k % NB_IN].ap(),
                    xa[4 * g : 4 * g + 4].rearrange(
                        "ci (pp r) w -> (ci pp) (r w)", pp=32
                    ),
                ).then_inc(ld_sems[k % NB_IN], 16)

        @block.vector
        def _(vec):
            for k in range(n):
                vec.wait_ge(ld_sems[k % NB_IN], 16 * (k // NB_IN + 1))
                if k >= NB_ST:
                    # staging reuse: stores of group k-NB_ST must be done
                    vec.wait_ge(st_sems[k % NB_ST], 64 * (k // NB_ST))
                t3 = tin[k % NB_IN].ap().rearrange("p (j w) -> p j w", j=16)
                s4 = (
                    tst[k % NB_ST]
                    .ap()
                    .rearrange("p (co hh w) -> p co hh w", co=4, hh=8)
                )
                last = None
                for dh in range(2):
                    for dw in range(2):
                        last = vec.tensor_copy(
                            s4[:, 2 * dh + dw], t3[:, dh::2, dw::2]
                        )
                last.then_inc(cp_sem, 1)

        @block.scalar
        def _(scalar):
            for k in range(n):
                g = k % G
                scalar.wait_ge(cp_sem, k + 1)
                s = tst[k % NB_ST].ap()
                for ci in range(4):
                    c0 = 16 * g + 4 * ci
                    scalar.dma_start(
                        oa[c0 : c0 + 4].rearrange(
                            "co (pp hh) w -> pp co (hh w)", hh=8
                        ),
                        s[32 * ci : 32 * ci + 32].rearrange(
                            "p (co q) -> p co q", co=4
                        ),
                    ).then_inc(st_sems[k % NB_ST], 16)


def _emit_v1(nc, tc, xa, oa, reps):
    """1 channel per tile: 1MB loads (8KB descs), 1MB stores (2KB descs)."""
    with (
        tc.tile_pool(name="inp", bufs=3) as ip,
        tc.tile_pool(name="stg", bufs=3) as sp,
    ):
        for _ in range(reps):
            for i in range(C):
                t = ip.tile([128, 2048], mybir.dt.float32)
                # partition p <- x[i, 4p:4p+4, :] (8KB contiguous per partition)
                nc.sync.dma_start(
                    t[:], xa[i].rearrange("(p r) w -> p (r w)", p=128)
                )
                s = sp.tile([128, 2048], mybir.dt.float32)
                t3 = t[:].rearrange("p (j w) -> p j w", j=4)
                s4 = s[:].rearrange("p (c hh w) -> p c hh w", c=4, hh=2)
                for dh in range(2):
                    for dw in range(2):
                        nc.vector.tensor_copy(
                            s4[:, 2 * dh + dw], t3[:, dh::2, dw::2]
                        )
                # staging partition p rows (2p, 2p+1) -> 2KB contiguous runs
                nc.sync.dma_start(
                    oa[4 * i : 4 * i + 4].rearrange(
                        "c (p hh) w -> p c (hh w)", p=128, hh=2
                    ),
                    s[:].rearrange("p (c q) -> p c q", c=4),
                )


def _emit_v2(nc, tc, xa, oa, reps, bufs, store_engine="scalar", store_sp=False, load_sp=False):
    """4 channels per tile (4MB): 8KB descriptors on BOTH load and store;
    loads on the SP HWDGE ring, stores on the ACT ring.

    Tile partition p = (ci=p>>5, pp=p&31) holds x[4g+ci, 16pp:16pp+16, :]
    (32KB contiguous).  Staging partition p holds, for each co in 0..3,
    out[4*(4g+ci)+co, 8pp:8pp+8, :] as one 8KB contiguous run.
    """
    G = C // 4  # 8 groups
    if isinstance(bufs, int):
        bufs = (bufs, bufs)
    with (
        tc.tile_pool(name="inp", bufs=bufs[0]) as ip,
        tc.tile_pool(name="stg", bufs=bufs[1]) as sp,
    ):
        for _ in range(reps):
            for g in range(G):
                t = ip.tile([128, 8192], mybir.dt.float32)
                nc.sync.dma_start(
                    t[:],
                    xa[4 * g : 4 * g + 4].rearrange(
                        "ci (pp r) w -> (ci pp) (r w)", pp=32
                    ),
                    single_packet=load_sp,
                )
                s = sp.tile([128, 8192], mybir.dt.float32)
                t3 = t[:].rearrange("p (j w) -> p j w", j=16)
                s4 = s[:].rearrange("p (co hh w) -> p co hh w", co=4, hh=8)
                for dh in range(2):
                    for dw in range(2):
                        nc.vector.tensor_copy(
                            s4[:, 2 * dh + dw], t3[:, dh::2, dw::2]
                        )
                for ci in range(4):
                    c0 = 16 * g + 4 * ci
                    if store_engine == "alt":
                        eng = nc.scalar if (4 * g + ci) % 2 == 0 else nc.sync
                    else:
                        eng = getattr(nc, store_engine)
                    eng.dma_start(
                        oa[c0 : c0 + 4].rearrange(
                            "co (pp hh) w -> pp co (hh w)", hh=8
                        ),
                        s[32 * ci : 32 * ci + 32].rearrange(
                            "p (co q) -> p co q", co=4
                        ),
                        single_packet=store_sp,
                    )


def _emit_v3(nc, tc, xa, oa, reps, bufs, store_engine="scalar", store_sp=True, load_sp=False, store_run=16384):
    """8 channels per tile (8MB load, 64KB/partition): doubles both the
    load and store descriptor sizes vs v2 (64KB loads, 16KB store runs)
    to amortize the ~115ns/descriptor DMA-engine overhead.

    Load partition p = (ci=p>>4, pp=p&15) holds x[8g+ci, 32pp:32pp+32, :]
    (64KB contiguous).  Staging is split in two dh-halves of 32KB/partition
    so SBUF fits (64KB*2 + 32KB*2 = 192KB/partition): half dh partition p
    holds, for dw in 0..1, out[4*(8g+ci)+2dh+dw, 16pp:16pp+16, :] as one
    16KB contiguous run.
    """
    G = C // 8  # 4 groups
    if isinstance(bufs, int):
        bufs = (bufs, bufs)
    store_eng = getattr(nc, store_engine)
    with (
        tc.tile_pool(name="inp", bufs=bufs[0]) as ip,
        tc.tile_pool(name="stg", bufs=bufs[1]) as sp,
    ):
        for _ in range(reps):
            for g in range(G):
                t = ip.tile([128, 16384], mybir.dt.float32)
                nc.sync.dma_start(
                    t[:],
                    xa[8 * g : 8 * g + 8].rearrange(
                        "ci (pp r) w -> (ci pp) (r w)", pp=16
                    ),
                    single_packet=load_sp,
                )
                t3 = t[:].rearrange("p (j w) -> p j w", j=32)
                for dh in range(2):
                    s = sp.tile([128, 8192], mybir.dt.float32)
                    s4 = s[:].rearrange("p (dw hh w) -> p dw hh w", dw=2, hh=16)
                    for dw in range(2):
                        nc.vector.tensor_copy(s4[:, dw], t3[:, dh::2, dw::2])
                    # (ci pp) isn't a single arithmetic stride in out, so
                    # one store per ci: 16 partitions x 2 runs of 16KB
                    # (or 2x2 runs of 8KB when store_run=8192)
                    for ci in range(8):
                        c0 = 4 * (8 * g + ci) + 2 * dh
                        if store_run == 16384:
                            store_eng.dma_start(
                                oa[c0 : c0 + 2].rearrange(
                                    "dw (pp hh) w -> pp dw (hh w)", hh=16
                                ),
                                s[16 * ci : 16 * ci + 16].rearrange(
                                    "p (dw q) -> p dw q", dw=2
                                ),
                                single_packet=store_sp,
                            )
                        else:
                            store_eng.dma_start(
                                oa[c0 : c0 + 2].rearrange(
                                    "dw (pp h2 hh) w -> pp h2 dw (hh w)",
                                    h2=2, hh=8,
                                ),
                                s[16 * ci : 16 * ci + 16].rearrange(
                                    "p (dw h2 q) -> p h2 dw q", dw=2, h2=2
                                ),
                                single_packet=store_sp,
                            )


def _emit_v4(nc, tc, xa, oa, reps, bufs, store_engine="scalar", store_sp=True, load_sp=False, store_run=16384):
    """Like v3 (64KB loads) but stores batched per (g, dh, dw)-phase:
    DRAM AP is 3 dims (ci=8 x 1MB, pp=16 x 16KB, run) against the full
    128-partition staging half, giving 128 descriptors per dma_start
    (8 per DMA engine) like v2, instead of v3's 2-deep batches.
    store_run=8192 splits each run in two via an extra h2 phase index.
    """
    G = C // 8  # 4 groups
    if isinstance(bufs, int):
        bufs = (bufs, bufs)
    store_eng = getattr(nc, store_engine)
    with (
        tc.tile_pool(name="inp", bufs=bufs[0]) as ip,
        tc.tile_pool(name="stg", bufs=bufs[1]) as sp,
    ):
        for _ in range(reps):
            for g in range(G):
                t = ip.tile([128, 16384], mybir.dt.float32)
                nc.sync.dma_start(
                    t[:],
                    xa[8 * g : 8 * g + 8].rearrange(
                        "ci (pp r) w -> (ci pp) (r w)", pp=16
                    ),
                    single_packet=load_sp,
                )
                t3 = t[:].rearrange("p (j w) -> p j w", j=32)
                og = oa[32 * g : 32 * g + 32]
                for dh in range(2):
                    s = sp.tile([128, 8192], mybir.dt.float32)
                    s4 = s[:].rearrange("p (dw hh w) -> p dw hh w", dw=2, hh=16)
                    for dw in range(2):
                        nc.vector.tensor_copy(s4[:, dw], t3[:, dh::2, dw::2])
                    if store_run == 16384:
                        od = og.rearrange(
                            "(ci co) (pp hh) w -> co ci pp (hh w)", co=4, hh=16
                        )
                        sd = s[:].rearrange("p (dw q) -> p dw q", dw=2)
                        for dw in range(2):
                            store_eng.dma_start(
                                od[2 * dh + dw], sd[:, dw],
                                single_packet=store_sp,
                            )
                    else:
                        od = og.rearrange(
                            "(ci co) (pp h2 hh) w -> co h2 ci pp (hh w)",
                            co=4, h2=2, hh=8,
                        )
                        sd = s[:].rearrange(
                            "p (dw h2 q) -> p dw h2 q", dw=2, h2=2
                        )
                        for dw in range(2):
                            for h2 in range(2):
                                store_eng.dma_start(
                                    od[2 * dh + dw, h2], sd[:, dw, h2],
                                    single_packet=store_sp,
                                )


def _emit_v5(nc, tc, xa, oa, reps, bufs, store_engine="scalar", store_sp=True, load_sp=False):
    """v4 with the partition map transposed: p = (pp<<3)|ci so the DRAM-side
    AP outer dim is pp=16 -> descriptors spread over all 16 DMA engines
    (engine = outer entry), each with an 8-deep chain (ci) of 16KB
    descriptors per dma_start.  At each chain step the 16 engines jointly
    cover one contiguous 256KB output channel (stores) / 1MB input channel
    (loads).
    """
    G = C // 8  # 4 groups
    if isinstance(bufs, int):
        bufs = (bufs, bufs)
    with (
        tc.tile_pool(name="inp", bufs=bufs[0]) as ip,
        tc.tile_pool(name="stg", bufs=bufs[1]) as sp,
    ):
        for _ in range(reps):
            for g in range(G):
                t = ip.tile([128, 16384], mybir.dt.float32)
                nc.sync.dma_start(
                    t[:],
                    xa[8 * g : 8 * g + 8].rearrange(
                        "ci (pp r) w -> pp ci (r w)", pp=16
                    ),
                    single_packet=load_sp,
                )
                t3 = t[:].rearrange("p (j w) -> p j w", j=32)
                od = oa[32 * g : 32 * g + 32].rearrange(
                    "(ci co) (pp hh) w -> co pp ci (hh w)", co=4, hh=16
                )
                for dh in range(2):
                    s = sp.tile([128, 8192], mybir.dt.float32)
                    s4 = s[:].rearrange("p (dw hh w) -> p dw hh w", dw=2, hh=16)
                    for dw in range(2):
                        nc.vector.tensor_copy(s4[:, dw], t3[:, dh::2, dw::2])
                    sd = s[:].rearrange("p (dw q) -> p dw q", dw=2)
                    for dw in range(2):
                        if store_engine == "alt":
                            eng = nc.scalar if dw == 0 else nc.sync
                        else:
                            eng = getattr(nc, store_engine)
                        eng.dma_start(
                            od[2 * dh + dw], sd[:, dw], single_packet=store_sp
                        )


def _emit_v7(nc, tc, xa, oa, reps, bufs, store_engine="scalar", store_sp=True, load_sp=False):
    """64KB loads (v4-style, merged outer, consecutive chains) + FULL
    64KB/partition staging (single tile per group) so stores can run
    per-ci with co as a 4-deep chain of 16KB descriptors on 16 engines.
    SBUF: 2x64KB load bufs + 1x64KB staging = 192KB/partition.
    """
    G = C // 8  # 4 groups
    if isinstance(bufs, int):
        bufs = (bufs, bufs)
    store_eng = getattr(nc, store_engine)
    with (
        tc.tile_pool(name="inp", bufs=bufs[0]) as ip,
        tc.tile_pool(name="stg", bufs=bufs[1]) as sp,
    ):
        for _ in range(reps):
            for g in range(G):
                t = ip.tile([128, 16384], mybir.dt.float32)
                nc.sync.dma_start(
                    t[:],
                    xa[8 * g : 8 * g + 8].rearrange(
                        "ci (pp r) w -> (ci pp) (r w)", pp=16
                    ),
                    single_packet=load_sp,
                )
                t3 = t[:].rearrange("p (j w) -> p j w", j=32)
                s = sp.tile([128, 16384], mybir.dt.float32)
                s4 = s[:].rearrange("p (co hh w) -> p co hh w", co=4, hh=16)
                for dh in range(2):
                    for dw in range(2):
                        nc.vector.tensor_copy(
                            s4[:, 2 * dh + dw], t3[:, dh::2, dw::2]
                        )
                for ci in range(8):
                    c0 = 4 * (8 * g + ci)
                    store_eng.dma_start(
                        oa[c0 : c0 + 4].rearrange(
                            "co (pp hh) w -> pp co (hh w)", hh=16
                        ),
                        s[16 * ci : 16 * ci + 16].rearrange(
                            "p (co q) -> p co q", co=4
                        ),
                        single_packet=store_sp,
                    )


def _emit_v8(nc, tc, xa, oa, reps, bufs, store_engine="scalar", store_sp=True, load_sp=False):
    """cal_mix's roofline shapes as a correct kernel.

    Partition map p = (pp<<3)|ci.  Loads are per-(g, ci): one 1MB channel
    as (pp[16]-outer, 64KB run) -> 16 engines, each one CONSECUTIVE 64KB
    descriptor (shallow chains are cheap for loads: completion acks go to
    SBUF, not HBM).  Stores are v5-shape: per (g, dh, dw) phase,
    (pp[16]-outer, ci 8-deep chain, 16KB runs) on 16 engines.
    """
    G = C // 8  # 4 groups
    if isinstance(bufs, int):
        bufs = (bufs, bufs)
    with (
        tc.tile_pool(name="inp", bufs=bufs[0]) as ip,
        tc.tile_pool(name="stg", bufs=bufs[1]) as sp,
    ):
        for _ in range(reps):
            for g in range(G):
                t = ip.tile([128, 16384], mybir.dt.float32)
                td = t[:].rearrange("(pp ci) x -> ci pp x", ci=8)
                for ci in range(8):
                    nc.sync.dma_start(
                        td[ci],
                        xa[8 * g + ci].rearrange("(pp r) w -> pp (r w)", pp=16),
                        single_packet=load_sp,
                    )
                t3 = t[:].rearrange("p (j w) -> p j w", j=32)
                od = oa[32 * g : 32 * g + 32].rearrange(
                    "(ci co) (pp hh) w -> co pp ci (hh w)", co=4, hh=16
                )
                for dh in range(2):
                    s = sp.tile([128, 8192], mybir.dt.float32)
                    s4 = s[:].rearrange("p (dw hh w) -> p dw hh w", dw=2, hh=16)
                    for dw in range(2):
                        nc.vector.tensor_copy(s4[:, dw], t3[:, dh::2, dw::2])
                    sd = s[:].rearrange("p (dw q) -> p dw q", dw=2)
                    for dw in range(2):
                        if store_engine == "alt":
                            eng = nc.scalar if dw == 0 else nc.sync
                        else:
                            eng = getattr(nc, store_engine)
                        eng.dma_start(
                            od[2 * dh + dw], sd[:, dw], single_packet=store_sp
                        )


def _emit_v9(nc, tc, xa, oa, reps, bufs, store_engine="scalar", store_sp=True, load_sp=False):
    """v4's private-slab loads + 16-engine 8-deep stores.

    Partition map p = (ci<<4)|pp (v4).  Loads: merged (ci pp)=128-outer,
    64KB consecutive descriptors, engine-private 512KB slabs (26+ GB/s).
    Stores: DRAM (pp[16]-outer, ci[8] chain, 16KB run); the SBUF source
    enumerates (pp, ci)-major via a partition-strided AP
    (pp stride-1, ci stride-16), so all 16 engines carry 8-deep chains.
    """
    G = C // 8  # 4 groups
    if isinstance(bufs, int):
        bufs = (bufs, bufs)
    with (
        tc.tile_pool(name="inp", bufs=bufs[0]) as ip,
        tc.tile_pool(name="stg", bufs=bufs[1]) as sp,
    ):
        for _ in range(reps):
            for g in range(G):
                t = ip.tile([128, 16384], mybir.dt.float32)
                nc.sync.dma_start(
                    t[:],
                    xa[8 * g : 8 * g + 8].rearrange(
                        "ci (pp r) w -> (ci pp) (r w)", pp=16
                    ),
                    single_packet=load_sp,
                )
                t3 = t[:].rearrange("p (j w) -> p j w", j=32)
                od = oa[32 * g : 32 * g + 32].rearrange(
                    "(ci co) (pp hh) w -> co pp ci (hh w)", co=4, hh=16
                )
                for dh in range(2):
                    s = sp.tile([128, 8192], mybir.dt.float32)
                    s4 = s[:].rearrange("p (dw hh w) -> p dw hh w", dw=2, hh=16)
                    for dw in range(2):
                        nc.vector.tensor_copy(s4[:, dw], t3[:, dh::2, dw::2])
                    sd = s[:].rearrange("(ci pp) (dw q) -> dw pp ci q", ci=8, dw=2)
                    for dw in range(2):
                        if store_engine == "alt":
                            eng = nc.scalar if dw == 0 else nc.sync
                        else:
                            eng = getattr(nc, store_engine)
                        eng.dma_start(
                            od[2 * dh + dw], sd[dw], single_packet=store_sp
                        )


def _emit_cal(nc, tc, xa, oa, reps, which="load"):
    """Calibration kernels (NOT correct output -- measure DMA roofline).

    load:   v4-shape loads only (merged 128-outer, consecutive chains)
    store:  v2-shape stores only (pp32-outer, co 4-deep chains, 8KB)
    store5: v5-shape stores only (pp16-outer, ci 8-deep chains, 16KB)
    mix:    v4-shape loads + v5-shape stores, fully independent
    """
    with (
        tc.tile_pool(name="inp", bufs=2) as ip,
        tc.tile_pool(name="stg", bufs=1) as sp,
    ):
        s = sp.tile([128, 8192], mybir.dt.float32)
        if which != "load":
            nc.vector.memset(s[:], 0.0)
        for _ in range(reps):
            if which == "store":
                for g in range(8):
                    for ci in range(4):
                        c0 = 16 * g + 4 * ci
                        nc.scalar.dma_start(
                            oa[c0 : c0 + 4].rearrange(
                                "co (pp hh) w -> pp co (hh w)", hh=8
                            ),
                            s[32 * ci : 32 * ci + 32].rearrange(
                                "p (co q) -> p co q", co=4
                            ),
                            single_packet=True,
                        )
                continue
            for g in range(4):
                if which in ("load", "mix"):
                    t = ip.tile([128, 16384], mybir.dt.float32)
                    nc.sync.dma_start(
                        t[:],
                        xa[8 * g : 8 * g + 8].rearrange(
                            "ci (pp r) w -> (ci pp) (r w)", pp=16
                        ),
                    )
                if which in ("store5", "mix"):
                    od = oa[32 * g : 32 * g + 32].rearrange(
                        "(ci co) (pp hh) w -> co pp ci (hh w)", co=4, hh=16
                    )
                    sd = s[:].rearrange("p (a q) -> p a q", a=2)
                    for co in range(4):
                        nc.scalar.dma_start(
                            od[co], sd[:, co % 2], single_packet=True
                        )


def kernel(x: np.ndarray) -> np.ndarray:
    assert x.shape == (B, C, H, W), x.shape
    if "nc" not in _cache:
        _cache["nc"] = _build_nc()
    nc = _cache["nc"]
    x = np.ascontiguousarray(np.asarray(x, dtype=np.float32))
    in_maps = [{"x": x[b]} for b in range(N_CORES)]
    res = run_bass_kernel_spmd(nc, in_maps, core_ids=list(range(N_CORES)))
    return np.stack([res.results[b]["out"] for b in range(N_CORES)], axis=0)

